# revision 16
# baseline (speedup 1.0000x reference)
"""Based linear-attention via chunked state form on 8 TRN2 NeuronCores.

Sharding: core c handles batch b = c // 4 and head-group g = c % 4
(3 of 12 heads).  Wq/Wk/Wv column-split by head, Wo row-split; each
core emits a partial [L, D] output and the host sums the 4 partials
per batch.

Algorithm: attn = 1 + s + 0.5 s^2 (s = q.k/sqrt(F)) is an exact
feature-map kernel phi(q).phi(k) with phi = [1, x, vec(x (x) x)]
(dim 1+16+256).  Chunked linear attention with C=128: per chunk the
intra part is one masked 128x128 quadratic block; the cross-chunk
part contracts phi(q) against a running state S = sum phi(k) (x)
[v | 1] per head ([2*17 + 256 rows, 129 cols]; column 128 carries
the causal normalizer z).  Everything is scaled 2x (attn2 = 2 + 2s
+ s^2 = (0.5*s2+1)^2 + 1 with s2 = 2s via Wq scaled by 0.5) so the
intra path stays one Square activation; the 2x cancels in o/z.

Matmul operands must sit at partition base 0/32/64 with equal bases
for lhsT/rhs, so qT/kT live in separate [96, L] tiles with head h in
rows 32h..32h+15; row 32h+16 holds the constant feature (2.0 in qT,
1.0 in kT), which merges the "1" feature into the 17-row ab state
block.  phi(q)'s 256 outer-product rows are built PE-side with 0/1
replication selectors + one DVE multiply (no psum->sbuf copy);
phi(k) gets l-major layout from one DMA transpose per chunk.
"""

import sys

sys.path.insert(0, "/opt/trn_rl_repo")

from contextlib import ExitStack

import ml_dtypes
import numpy as np

import concourse.bass as bass
import concourse.tile as tile
from concourse import bacc, mybir
from concourse.bass_utils import run_bass_kernel_spmd
from concourse.masks import make_identity

B, L, D = 2, 2048, 1536
H, FDIM, HD = 12, 16, 128
NH = 3            # heads per core
P = 128
NK = D // P       # 12 contraction tiles
NCH = L // P      # 16 chunks
GQ = 96           # padded q rows (3 heads x 32); same for k
DV = NH * HD      # 384 v cols per core
DVA = HD + 1      # 129: v columns + ones column per head

DT = mybir.dt.bfloat16
NPDT = ml_dtypes.bfloat16
F32 = mybir.dt.float32

_ADD = mybir.AluOpType.add
_MULT = mybir.AluOpType.mult
_SQUARE = mybir.ActivationFunctionType.Square
_COPY = mybir.ActivationFunctionType.Copy


def _build():
    nc = bacc.Bacc("TRN2", target_bir_lowering=False, debug=False, num_devices=8)

    hsT = nc.dram_tensor("hsT", [D, L], DT, kind="ExternalInput").ap()
    wqk = nc.dram_tensor("wqk", [D, 2 * GQ], DT, kind="ExternalInput").ap()
    wv = nc.dram_tensor("wv", [D, DV], DT, kind="ExternalInput").ap()
    wo = nc.dram_tensor("wo", [DV, D], DT, kind="ExternalInput").ap()
    maskd = nc.dram_tensor("maskd", [P, P], DT, kind="ExternalInput").ap()
    rseld = nc.dram_tensor("rseld", [GQ, 3 * P], DT, kind="ExternalInput").ap()
    out = nc.dram_tensor("out", [L, D], DT, kind="ExternalOutput").ap()
    dbg = nc.dram_tensor("dbg", [P, 2048], DT, kind="ExternalOutput").ap()

    with tile.TileContext(nc, trace_sim=False) as tc, ExitStack() as ctx:
        cpool = ctx.enter_context(tc.tile_pool(name="consts", bufs=1))
        wqk_sb = cpool.tile([P, NK * 2 * GQ], DT, tag="wqk")
        wv_sb = cpool.tile([P, NK * DV], DT, tag="wv")
        wo_sb = cpool.tile([P, NH * D], DT, tag="wo")
        mask_sb = cpool.tile([P, P], DT, tag="mask")
        rsel_sb = cpool.tile([GQ, 3 * P], DT, tag="rsel")
        ident = cpool.tile([P, P], F32, tag="ident")
        hs_all = cpool.tile([P, NK * L], DT, tag="hs")     # [p, (k l)]
        qT_sb = cpool.tile([GQ, L], DT, tag="qT")
        kT_sb = cpool.tile([GQ, L], DT, tag="kT")
        v_sb = cpool.tile([P, NCH * NH * DVA], DT, tag="v")  # [l, (c h dv)]
        S_sb = [[cpool.tile([P, NH * DVA], DT, tag=f"S{u}{h}", name=f"S{u}{h}")
                 for h in range(NH)] for u in range(2)]

        # ---- input DMA (few big ops; strip 0 first so chunk 0 can start)
        nc.sync.dma_start(wqk_sb[:].rearrange("p (k g) -> p k g", k=NK),
                          wqk.rearrange("(k p) g -> p k g", p=P))
        SW = 4 * P
        for s in range(4):
            nc.sync.dma_start(
                hs_all[:].rearrange("p (k l) -> p k l", k=NK)[:, :, s * SW:(s + 1) * SW],
                hsT.rearrange("(k p) l -> p k l", p=P)[:, :, s * SW:(s + 1) * SW])
            if s == 0:
                nc.sync.dma_start(mask_sb[:], maskd)
                nc.sync.dma_start(rsel_sb[:], rseld)
                nc.sync.dma_start(wv_sb[:].rearrange("p (k g) -> p k g", k=NK),
                                  wv.rearrange("(k p) g -> p k g", p=P))
            elif s == 1:
                nc.sync.dma_start(wo_sb[:].rearrange("p (h d) -> p h d", h=NH),
                                  wo.rearrange("(h p) d -> p h d", p=P))
        make_identity(nc, ident[:])
        # constant feature rows: 2.0 in qT (the "2" of attn2), 1.0 in kT.
        # Whole 32-row groups (partition alignment); the real q/k rows are
        # overwritten by the projection copies before any read.
        for h in range(NH):
            nc.vector.memset(qT_sb[32 * h: 32 * h + 32, :], 2.0)
            nc.vector.memset(kT_sb[32 * h: 32 * h + 32, :], 1.0)
        # ones column of v_aug, all chunks/heads at once
        nc.vector.memset(
            v_sb[:].rearrange("p (c h x) -> p c h x", c=NCH, h=NH)[:, :, :, HD], 1.0)
        for u in range(2):
            for h in range(NH):
                nc.vector.memset(S_sb[u][h][:], 0.0)

        # ---- PSUM: exactly 8 banks
        ppool = ctx.enter_context(tc.tile_pool(name="ps", bufs=1, space="PSUM"))
        pb1 = ppool.tile([P, 512], F32, tag="pb1")   # qk + slots A,B,E1
        pb2 = ppool.tile([P, 512], F32, tag="pb2")   # v + slots C,C2,E2
        S_ps = [ppool.tile([P, NH * DVA], F32, tag=f"Sp{h}", name=f"Sp{h}")
                for h in range(NH)]                   # 3 banks
        o_ps = ppool.tile([P, NH * DVA], F32, tag="o_ps")  # 1 bank
        op_ps = [ppool.tile([P, 512], F32, tag=f"op{i}", name=f"op{i}")
                 for i in range(2)]                   # oproj double buffer

        qk_ps = pb1[0:GQ, 0:128]
        slotA = pb1[:, 128:256]
        slotB = pb1[:, 256:384]
        slotE1 = pb1[:, 384:512]
        v_ps = pb2[:, 0:128]
        slotC = pb2[:, 128:256]
        slotC2 = pb2[:, 256:384]
        slotE2 = pb2[:, 384:512]
        sT_slots = [slotA, slotB, slotC]
        q2_slots = [[slotC2, slotE1, slotE2], [slotA, slotB, slotC],
                    [slotC2, slotE1, slotE2]]
        oT_slots = [slotC2, slotE1, slotE2]

        # dead regions of the S psum ab-block must read as 0
        for h in range(NH):
            nc.vector.memset(S_ps[h][:, 0:DVA], 0.0)

        wpool = ctx.enter_context(tc.tile_pool(name="work", bufs=1))
        attT = [[wpool.tile([P, P], DT, tag=f"attT{u}{h}", name=f"attT{u}{h}")
                 for h in range(NH)] for u in range(2)]
        fq2T = [[[wpool.tile([P, P], DT, tag=f"fq2T{u}{h}{a}", name=f"fq2T{u}{h}{a}")
                  for a in range(2)] for h in range(NH)] for u in range(2)]
        fk2 = [[wpool.tile([P, 2 * P], DT, tag=f"fk2{u}{h}", name=f"fk2{u}{h}")
                for h in range(NH)] for u in range(2)]
        k_rm = [wpool.tile([P, GQ], DT, tag=f"krm{u}", name=f"krm{u}")
                for u in range(2)]
        rep_sb = [[wpool.tile([P, P], DT, tag=f"rep{u}{h}", name=f"rep{u}{h}")
                   for h in range(NH)] for u in range(2)]
        o_sb = [[wpool.tile([P, P], F32, tag=f"o{u}{h}", name=f"o{u}{h}")
                 for h in range(NH)] for u in range(2)]
        oT_sb = [[wpool.tile([P, P], DT, tag=f"oT{u}{h}", name=f"oT{u}{h}")
                  for h in range(NH)] for u in range(2)]
        zinv = [wpool.tile([P, NH], F32, tag=f"zi{u}", name=f"zi{u}")
                for u in range(2)]
        out_sb = [wpool.tile([P, D], DT, tag=f"out{u}", name=f"out{u}")
                  for u in range(2)]
        dbg_f32 = wpool.tile([P, P + NH], DT, tag="dbgf", name="dbgf")

        def proj_qk(c, which):
            """project qT (which=0) or kT (which=1) columns for chunk c."""
            dst = qT_sb if which == 0 else kT_sb
            for k in range(NK):
                nc.tensor.matmul(
                    qk_ps,
                    wqk_sb[:, k * 2 * GQ + which * GQ: k * 2 * GQ + (which + 1) * GQ],
                    hs_all[:, k * L + c * P: k * L + (c + 1) * P],
                    start=(k == 0), stop=(k == NK - 1))
            for h in range(NH):  # skip pad rows (row 32h+16 holds the const)
                nc.scalar.activation(dst[32 * h:32 * h + FDIM, c * P:(c + 1) * P],
                                     qk_ps[32 * h:32 * h + FDIM, :], _COPY)

        def vproj_third(c, t):
            for k in range(NK):
                nc.tensor.matmul(
                    v_ps, hs_all[:, k * L + c * P: k * L + (c + 1) * P],
                    wv_sb[:, k * DV + t * HD: k * DV + (t + 1) * HD],
                    start=(k == 0), stop=(k == NK - 1))
            nc.scalar.activation(
                v_sb[:, (c * NH + t) * DVA: (c * NH + t) * DVA + HD], v_ps, _COPY)

        def q2_trio(c, h):
            """PE replication of q2T rows for head h, then DVE outer products."""
            u = c % 2
            q2 = qT_sb[32 * h:32 * h + FDIM, c * P:(c + 1) * P]
            rep, expA, expB = q2_slots[h]
            nc.tensor.matmul(rep, rsel_sb[32 * h:32 * h + FDIM, 0:P], q2,
                             start=True, stop=True)
            nc.tensor.matmul(expA, rsel_sb[32 * h:32 * h + FDIM, P:2 * P], q2,
                             start=True, stop=True)
            nc.tensor.matmul(expB, rsel_sb[32 * h:32 * h + FDIM, 2 * P:3 * P], q2,
                             start=True, stop=True)
            rs = rep_sb[u][h]
            nc.vector.tensor_copy(rs[:], rep)
            nc.vector.scalar_tensor_tensor(fq2T[u][h][0][:], expA, 0.25, rs[:],
                                           op0=_MULT, op1=_MULT)
            nc.vector.scalar_tensor_tensor(fq2T[u][h][1][:], expB, 0.25, rs[:],
                                           op0=_MULT, op1=_MULT)

        def fk2_mults(c):
            u = c % 2
            for h in range(NH):
                krm = k_rm[u][:, 32 * h:32 * h + FDIM]
                nc.vector.tensor_mul(
                    fk2[u][h][:].rearrange("p (i j) -> p i j", i=FDIM),
                    krm[:, :, None].broadcast_to([P, FDIM, FDIM]),
                    krm[:, None, :].broadcast_to([P, FDIM, FDIM]))

        def attention(c):
            u = c % 2
            for h in range(NH):
                nc.tensor.matmul(
                    sT_slots[h],
                    kT_sb[32 * h:32 * h + FDIM, c * P:(c + 1) * P],
                    qT_sb[32 * h:32 * h + FDIM, c * P:(c + 1) * P],
                    start=True, stop=True)
                nc.scalar.activation(attT[u][h][:], sT_slots[h], _SQUARE,
                                     bias=1.0, scale=0.5)
                nc.vector.scalar_tensor_tensor(attT[u][h][:], attT[u][h][:], 1.0,
                                               mask_sb[:], op0=_ADD, op1=_MULT)
            # S updates first: independent of attT, covers the act/mask latency
            for h in range(NH):
                va = v_sb[:, (c * NH + h) * DVA:(c * NH + h + 1) * DVA]
                r0 = 32 * h
                nc.tensor.matmul(S_ps[h][r0:r0 + 17, 0:DVA],
                                 k_rm[u][:, r0:r0 + 17], va,
                                 start=True, stop=True)
                nc.tensor.matmul(S_ps[h][0:P, DVA:2 * DVA],
                                 fk2[u][h][:, 0:P], va, start=True, stop=True)
                nc.tensor.matmul(S_ps[h][0:P, 2 * DVA:3 * DVA],
                                 fk2[u][h][:, P:2 * P], va, start=True, stop=True)
            # one CONTIGUOUS accumulation group per head: a start=True
            # re-arms the whole 2KB psum zero region, so groups sharing the
            # o_ps bank must never interleave
            sp = (c - 1) % 2
            for h in range(NH):
                og = o_ps[:, h * DVA:(h + 1) * DVA]
                r0 = 32 * h
                fa, fb = fq2T[u][h]
                if c > 0:
                    nc.tensor.matmul(og, qT_sb[r0:r0 + 17, c * P:(c + 1) * P],
                                     S_sb[sp][h][r0:r0 + 17, 0:DVA],
                                     start=True, stop=False)
                    nc.tensor.matmul(og, fa[:], S_sb[sp][h][0:P, DVA:2 * DVA],
                                     start=False, stop=False)
                    nc.tensor.matmul(og, fb[:], S_sb[sp][h][0:P, 2 * DVA:3 * DVA],
                                     start=False, stop=False)
                nc.tensor.matmul(og, attT[u][h][:],
                                 v_sb[:, (c * NH + h) * DVA:(c * NH + h + 1) * DVA],
                                 start=(c == 0), stop=True)

        def s_accum(c):
            with nc.allow_low_precision(reason="bf16 state accumulate, 2e-2 gate"):
                for h in range(NH):
                    nc.vector.tensor_add(S_sb[c % 2][h][:], S_ps[h][:],
                                         S_sb[(c - 1) % 2][h][:])

        def normalize(c):
            u = c % 2
            zc = o_ps.rearrange("p (h x) -> p h x", h=NH)[:, :, HD]
            nc.vector.reciprocal(zinv[u][:], zc)
            for h in range(NH):
                nc.scalar.activation(o_sb[u][h][:], o_ps[:, h * DVA: h * DVA + HD],
                                     _COPY, scale=zinv[u][:, h:h + 1])

        def o_transpose(c):
            u = c % 2
            for h in range(NH):
                nc.tensor.transpose(oT_slots[h], o_sb[u][h][:], ident[:])
                with nc.allow_low_precision(reason="oT copy bf16"):
                    nc.vector.tensor_copy(oT_sb[u][h][:], oT_slots[h])

        def oproj(c):
            u = c % 2
            for dc in range(3):
                ops = op_ps[(c * 3 + dc) % 2]
                for h in range(NH):
                    nc.tensor.matmul(ops, oT_sb[u][h][:],
                                     wo_sb[:, h * D + dc * 512: h * D + (dc + 1) * 512],
                                     start=(h == 0), stop=(h == NH - 1))
                nc.scalar.activation(out_sb[u][:, dc * 512:(dc + 1) * 512], ops, _COPY)
            nc.sync.dma_start(out[c * P:(c + 1) * P, :], out_sb[u][:])

        def prep(c):
            """everything chunk c needs, interleaved for PE continuity."""
            proj_qk(c, 0)
            q2_trio(c, 0)
            vproj_third(c, 0)
            q2_trio(c, 1)
            vproj_third(c, 1)
            q2_trio(c, 2)
            proj_qk(c, 1)
            nc.sync.dma_start_transpose(k_rm[c % 2][:],
                                        kT_sb[:, c * P:(c + 1) * P])
            vproj_third(c, 2)
            fk2_mults(c)

        # ---- main loop
        prep(0)
        for c in range(NCH):
            attention(c)
            if c < NCH - 1:
                prep(c + 1)
            s_accum(c)
            normalize(c)
            if c > 0:
                oproj(c - 1)
            o_transpose(c)
        oproj(NCH - 1)

    nc.compile()
    return nc


def _host_inputs(hidden_states, Wq, Wk, Wv, Wo):
    """Shard + lay out the full inputs into 8 per-core in_maps."""
    mask = (np.arange(P)[:, None] <= np.arange(P)[None, :]).astype(np.float32)

    # replication selectors, copies at row offsets 0/32/64 (one per head base)
    rsel = np.zeros((GQ, 3 * P), dtype=np.float32)
    for r0 in (0, 32, 64):
        for i in range(8):
            for j in range(FDIM):
                rsel[r0 + j, FDIM * i + j] = 1.0            # rep <- q2[j]
                rsel[r0 + i, P + FDIM * i + j] = 1.0        # expA <- q2[i]
                rsel[r0 + 8 + i, 2 * P + FDIM * i + j] = 1.0  # expB <- q2[i+8]

    in_maps = []
    for core in range(8):
        b, g = divmod(core, 4)
        heads = range(NH * g, NH * (g + 1))
        wqk_pack = np.zeros((D, 2 * GQ), dtype=np.float32)
        for i, h in enumerate(heads):
            wqk_pack[:, 32 * i: 32 * i + FDIM] = Wq[:, FDIM * h: FDIM * (h + 1)] * 0.5
            wqk_pack[:, GQ + 32 * i: GQ + 32 * i + FDIM] = \
                Wk[:, FDIM * h: FDIM * (h + 1)]
        in_maps.append({
            "hsT": np.ascontiguousarray(hidden_states[b].T).astype(NPDT),
            "wqk": wqk_pack.astype(NPDT),
            "wv": np.ascontiguousarray(Wv[:, HD * NH * g: HD * NH * (g + 1)]).astype(NPDT),
            "wo": np.ascontiguousarray(Wo[HD * NH * g: HD * NH * (g + 1), :]).astype(NPDT),
            "maskd": mask.astype(NPDT),
            "rseld": rsel.astype(NPDT),
        })
    return in_maps


_NC = None


def _get_nc():
    global _NC
    if _NC is None:
        _NC = _build()
    return _NC


def run(hidden_states, Wq, Wk, Wv, Wo, trace=False, **trace_kwargs):
    nc = _get_nc()
    in_maps = _host_inputs(hidden_states, Wq, Wk, Wv, Wo)
    res = run_bass_kernel_spmd(nc, in_maps, core_ids=list(range(8)),
                               trace=trace, **trace_kwargs)
    out = np.zeros((B, L, D), dtype=np.float32)
    for core in range(8):
        out[core // 4] += res.results[core]["out"].astype(np.float32)
    return out, res


def kernel(hidden_states, Wq, Wk, Wv, Wo):
    out, _ = run(np.asarray(hidden_states, dtype=np.float32),
                 np.asarray(Wq, dtype=np.float32),
                 np.asarray(Wk, dtype=np.float32),
                 np.asarray(Wv, dtype=np.float32),
                 np.asarray(Wo, dtype=np.float32))
    return out


# revision 19
# speedup vs baseline: 1.1550x; 1.1550x over previous
"""Based linear-attention via chunked state form on 8 TRN2 NeuronCores.

Sharding: core c handles batch b = c // 4 and head-group g = c % 4
(3 of 12 heads).  Wq/Wk/Wv column-split by head, Wo row-split; each
core emits a partial [L, D] output and the host sums the 4 partials
per batch.

Algorithm: attn = 1 + s + 0.5 s^2 (s = q.k/sqrt(F)) is an exact
feature-map kernel phi(q).phi(k) with phi = [1, x, vec(x (x) x)]
(dim 1+16+256).  Chunked linear attention with C=128: per chunk the
intra part is one masked 128x128 quadratic block; the cross-chunk
part contracts phi(q) against a running state S = sum phi(k) (x)
[v | 1] per head ([2*17 + 256 rows, 129 cols]; column 128 carries
the causal normalizer z).  Everything is scaled 2x (attn2 = 2 + 2s
+ s^2 = (0.5*s2+1)^2 + 1 with s2 = 2s via Wq scaled by 0.5) so the
intra path stays one Square activation; the 2x cancels in o/z.

Matmul operands must sit at partition base 0/32/64 with equal bases
for lhsT/rhs, so qT/kT live in separate [96, L] tiles with head h in
rows 32h..32h+15; row 32h+16 holds the constant feature (2.0 in qT,
1.0 in kT), which merges the "1" feature into the 17-row ab state
block.  phi(q)'s 256 outer-product rows are built PE-side with 0/1
replication selectors + one DVE multiply (no psum->sbuf copy);
phi(k) gets l-major layout from one DMA transpose per chunk.
"""

import sys

sys.path.insert(0, "/opt/trn_rl_repo")

from contextlib import ExitStack

import ml_dtypes
import numpy as np

import concourse.bass as bass
import concourse.tile as tile
from concourse import bacc, mybir
from concourse.bass_utils import run_bass_kernel_spmd
from concourse.masks import make_identity

B, L, D = 2, 2048, 1536
H, FDIM, HD = 12, 16, 128
NH = 3            # heads per core
P = 128
NK = D // P       # 12 contraction tiles
NCH = L // P      # 16 chunks
GQ = 96           # padded q rows (3 heads x 32); same for k
DV = NH * HD      # 384 v cols per core
DVA = HD + 1      # 129: v columns + ones column per head

DT = mybir.dt.bfloat16
NPDT = ml_dtypes.bfloat16
F32 = mybir.dt.float32

_ADD = mybir.AluOpType.add
_MULT = mybir.AluOpType.mult
_SQUARE = mybir.ActivationFunctionType.Square
_COPY = mybir.ActivationFunctionType.Copy


def _build():
    nc = bacc.Bacc("TRN2", target_bir_lowering=False, debug=False, num_devices=8)

    hsT = nc.dram_tensor("hsT", [D, L], DT, kind="ExternalInput").ap()
    wqk = nc.dram_tensor("wqk", [D, 2 * GQ], DT, kind="ExternalInput").ap()
    wv = nc.dram_tensor("wv", [D, DV], DT, kind="ExternalInput").ap()
    wo = nc.dram_tensor("wo", [DV, D], DT, kind="ExternalInput").ap()
    maskd = nc.dram_tensor("maskd", [P, P], DT, kind="ExternalInput").ap()
    rseld = nc.dram_tensor("rseld", [GQ, 3 * P + 2], DT, kind="ExternalInput").ap()
    out = nc.dram_tensor("out", [L, D], DT, kind="ExternalOutput").ap()
    dbg = nc.dram_tensor("dbg", [P, 2048], DT, kind="ExternalOutput").ap()

    with tile.TileContext(nc, trace_sim=False) as tc, ExitStack() as ctx:
        cpool = ctx.enter_context(tc.tile_pool(name="consts", bufs=1))
        wqk_sb = cpool.tile([P, NK * 2 * GQ], DT, tag="wqk")
        wv_sb = cpool.tile([P, NK * DV], DT, tag="wv")
        wo_sb = cpool.tile([P, NH * D], DT, tag="wo")
        mask_sb = cpool.tile([P, P], DT, tag="mask")
        rsel_sb = cpool.tile([GQ, 3 * P + 2], DT, tag="rsel")
        ident = cpool.tile([P, P], F32, tag="ident")
        hs_all = cpool.tile([P, NK * L], DT, tag="hs")     # [p, (k l)]
        qT_sb = cpool.tile([GQ, L], DT, tag="qT")
        kT_sb = cpool.tile([GQ, L], DT, tag="kT")
        v_sb = cpool.tile([P, NCH * NH * DVA], DT, tag="v")  # [l, (c h dv)]
        S_sb = [[cpool.tile([P, NH * DVA], DT, tag=f"S{u}{h}", name=f"S{u}{h}")
                 for h in range(NH)] for u in range(2)]

        # ---- input DMA (few big ops; strip 0 first so chunk 0 can start)
        nc.sync.dma_start(wqk_sb[:].rearrange("p (k g) -> p k g", k=NK),
                          wqk.rearrange("(k p) g -> p k g", p=P))
        SW = 4 * P
        for s in range(4):
            nc.sync.dma_start(
                hs_all[:].rearrange("p (k l) -> p k l", k=NK)[:, :, s * SW:(s + 1) * SW],
                hsT.rearrange("(k p) l -> p k l", p=P)[:, :, s * SW:(s + 1) * SW])
            if s == 0:
                nc.sync.dma_start(mask_sb[:], maskd)
                nc.sync.dma_start(rsel_sb[:], rseld)
                nc.sync.dma_start(wv_sb[:].rearrange("p (k g) -> p k g", k=NK),
                                  wv.rearrange("(k p) g -> p k g", p=P))
            elif s == 1:
                nc.sync.dma_start(wo_sb[:].rearrange("p (h d) -> p h d", h=NH),
                                  wo.rearrange("(h p) d -> p h d", p=P))
        make_identity(nc, ident[:])
        # constant feature rows: 2.0 in qT (the "2" of attn2), 1.0 in kT.
        # Whole 32-row groups (partition alignment); the real q/k rows are
        # overwritten by the projection copies before any read.
        for h in range(NH):
            nc.vector.memset(qT_sb[32 * h: 32 * h + 32, :], 2.0)
            nc.vector.memset(kT_sb[32 * h: 32 * h + 32, :], 1.0)
        # ones column of v_aug, all chunks/heads at once
        nc.vector.memset(
            v_sb[:].rearrange("p (c h x) -> p c h x", c=NCH, h=NH)[:, :, :, HD], 1.0)
        for u in range(2):
            for h in range(NH):
                nc.vector.memset(S_sb[u][h][:], 0.0)

        # ---- PSUM: exactly 8 banks
        ppool = ctx.enter_context(tc.tile_pool(name="ps", bufs=1, space="PSUM"))
        pb1 = ppool.tile([P, 512], F32, tag="pb1")   # qk + slots A,B,E1
        pb2 = ppool.tile([P, 512], F32, tag="pb2")   # v + slots C,C2,E2
        S_ps = [ppool.tile([P, NH * DVA], F32, tag=f"Sp{h}", name=f"Sp{h}")
                for h in range(NH)]                   # 3 banks
        o_ps = ppool.tile([P, NH * DVA], F32, tag="o_ps")  # 1 bank
        op_ps = [ppool.tile([P, 512], F32, tag=f"op{i}", name=f"op{i}")
                 for i in range(2)]                   # oproj double buffer

        qk_ps = pb1[0:GQ, 0:128]
        slotA = pb1[:, 128:256]
        slotB = pb1[:, 256:384]
        slotE1 = pb1[:, 384:512]
        v_ps = pb2[:, 0:128]
        slotC = pb2[:, 128:256]
        slotC2 = pb2[:, 256:384]
        slotE2 = pb2[:, 384:512]
        sT_slots = [slotA, slotB, slotC]
        q2_slots = [[slotC2, slotE1, slotE2], [slotA, slotB, slotC],
                    [slotC2, slotE1, slotE2]]
        oT_slots = [slotC2, slotE1, slotE2]

        # dead regions of the S psum ab-block must read as 0
        for h in range(NH):
            nc.vector.memset(S_ps[h][:, 0:DVA], 0.0)

        wpool = ctx.enter_context(tc.tile_pool(name="work", bufs=1))
        attT = [[wpool.tile([P, P], DT, tag=f"attT{u}{h}", name=f"attT{u}{h}")
                 for h in range(NH)] for u in range(2)]
        fq2T = [[[wpool.tile([P, P], DT, tag=f"fq2T{u}{h}{a}", name=f"fq2T{u}{h}{a}")
                  for a in range(2)] for h in range(NH)] for u in range(2)]
        fk2 = [[wpool.tile([P, 2 * P], DT, tag=f"fk2{u}{h}", name=f"fk2{u}{h}")
                for h in range(NH)] for u in range(2)]
        k_rm = [wpool.tile([P, GQ], DT, tag=f"krm{u}", name=f"krm{u}")
                for u in range(2)]
        rep_sb = [[wpool.tile([P, P], DT, tag=f"rep{u}{h}", name=f"rep{u}{h}")
                   for h in range(NH)] for u in range(2)]
        o_sb = [[wpool.tile([P, P], F32, tag=f"o{u}{h}", name=f"o{u}{h}")
                 for h in range(NH)] for u in range(2)]
        oT_sb = [[wpool.tile([P, P], DT, tag=f"oT{u}{h}", name=f"oT{u}{h}")
                  for h in range(NH)] for u in range(2)]
        zinv = [wpool.tile([P, NH], F32, tag=f"zi{u}", name=f"zi{u}")
                for u in range(2)]
        out_sb = [wpool.tile([P, D], DT, tag=f"out{u}", name=f"out{u}")
                  for u in range(2)]
        dbg_f32 = wpool.tile([P, P + NH], DT, tag="dbgf", name="dbgf")

        def proj_qk(c, which):
            """project qT (which=0) or kT (which=1) columns for chunk c."""
            dst = qT_sb if which == 0 else kT_sb
            for k in range(NK):
                nc.tensor.matmul(
                    qk_ps,
                    wqk_sb[:, k * 2 * GQ + which * GQ: k * 2 * GQ + (which + 1) * GQ],
                    hs_all[:, k * L + c * P: k * L + (c + 1) * P],
                    start=(k == 0), stop=(k == NK - 1))
            bias = rsel_sb[:, 3 * P + which: 3 * P + which + 1]
            nc.scalar.activation(dst[:, c * P:(c + 1) * P], qk_ps,
                                 mybir.ActivationFunctionType.Identity, bias=bias)

        def vproj_third(c, t):
            for k in range(NK):
                nc.tensor.matmul(
                    v_ps, hs_all[:, k * L + c * P: k * L + (c + 1) * P],
                    wv_sb[:, k * DV + t * HD: k * DV + (t + 1) * HD],
                    start=(k == 0), stop=(k == NK - 1))
            nc.scalar.activation(
                v_sb[:, (c * NH + t) * DVA: (c * NH + t) * DVA + HD], v_ps, _COPY)

        def q2_trio(c, h):
            """PE replication of q2T rows for head h, then DVE outer products."""
            u = c % 2
            q2 = qT_sb[32 * h:32 * h + FDIM, c * P:(c + 1) * P]
            rep, expA, expB = q2_slots[h]
            nc.tensor.matmul(rep, rsel_sb[32 * h:32 * h + FDIM, 0:P], q2,
                             start=True, stop=True)
            nc.tensor.matmul(expA, rsel_sb[32 * h:32 * h + FDIM, P:2 * P], q2,
                             start=True, stop=True)
            nc.tensor.matmul(expB, rsel_sb[32 * h:32 * h + FDIM, 2 * P:3 * P], q2,
                             start=True, stop=True)
            rs = rep_sb[u][h]
            nc.vector.tensor_copy(rs[:], rep)
            nc.vector.scalar_tensor_tensor(fq2T[u][h][0][:], expA, 0.25, rs[:],
                                           op0=_MULT, op1=_MULT)
            nc.vector.scalar_tensor_tensor(fq2T[u][h][1][:], expB, 0.25, rs[:],
                                           op0=_MULT, op1=_MULT)

        def fk2_mults(c):
            u = c % 2
            for h in range(NH):
                krm = k_rm[u][:, 32 * h:32 * h + FDIM]
                nc.gpsimd.tensor_mul(
                    fk2[u][h][:].rearrange("p (i j) -> p i j", i=FDIM),
                    krm[:, :, None].broadcast_to([P, FDIM, FDIM]),
                    krm[:, None, :].broadcast_to([P, FDIM, FDIM]))

        def attention(c):
            u = c % 2
            for h in range(NH):
                nc.tensor.matmul(
                    sT_slots[h],
                    kT_sb[32 * h:32 * h + FDIM, c * P:(c + 1) * P],
                    qT_sb[32 * h:32 * h + FDIM, c * P:(c + 1) * P],
                    start=True, stop=True)
                nc.scalar.activation(attT[u][h][:], sT_slots[h], _SQUARE,
                                     bias=1.0, scale=0.5)
                nc.vector.scalar_tensor_tensor(attT[u][h][:], attT[u][h][:], 1.0,
                                               mask_sb[:], op0=_ADD, op1=_MULT)
            # S updates first: independent of attT, covers the act/mask latency
            for h in range(NH):
                va = v_sb[:, (c * NH + h) * DVA:(c * NH + h + 1) * DVA]
                r0 = 32 * h
                nc.tensor.matmul(S_ps[h][r0:r0 + 17, 0:DVA],
                                 k_rm[u][:, r0:r0 + 17], va,
                                 start=True, stop=True)
                nc.tensor.matmul(S_ps[h][0:P, DVA:2 * DVA],
                                 fk2[u][h][:, 0:P], va, start=True, stop=True)
                nc.tensor.matmul(S_ps[h][0:P, 2 * DVA:3 * DVA],
                                 fk2[u][h][:, P:2 * P], va, start=True, stop=True)
            # one CONTIGUOUS accumulation group per head: a start=True
            # re-arms the whole 2KB psum zero region, so groups sharing the
            # o_ps bank must never interleave
            sp = (c - 1) % 2
            for h in range(NH):
                og = o_ps[:, h * DVA:(h + 1) * DVA]
                r0 = 32 * h
                fa, fb = fq2T[u][h]
                if c > 0:
                    nc.tensor.matmul(og, qT_sb[r0:r0 + 17, c * P:(c + 1) * P],
                                     S_sb[sp][h][r0:r0 + 17, 0:DVA],
                                     start=True, stop=False)
                    nc.tensor.matmul(og, fa[:], S_sb[sp][h][0:P, DVA:2 * DVA],
                                     start=False, stop=False)
                    nc.tensor.matmul(og, fb[:], S_sb[sp][h][0:P, 2 * DVA:3 * DVA],
                                     start=False, stop=False)
                nc.tensor.matmul(og, attT[u][h][:],
                                 v_sb[:, (c * NH + h) * DVA:(c * NH + h + 1) * DVA],
                                 start=(c == 0), stop=True)

        def s_accum(c):
            with nc.allow_low_precision(reason="bf16 state accumulate, 2e-2 gate"):
                for h in range(NH):
                    nc.vector.tensor_add(S_sb[c % 2][h][:], S_ps[h][:],
                                         S_sb[(c - 1) % 2][h][:])

        def normalize(c):
            u = c % 2
            zc = o_ps.rearrange("p (h x) -> p h x", h=NH)[:, :, HD]
            nc.vector.reciprocal(zinv[u][:], zc)
            for h in range(NH):
                nc.scalar.activation(o_sb[u][h][:], o_ps[:, h * DVA: h * DVA + HD],
                                     _COPY, scale=zinv[u][:, h:h + 1])

        def o_transpose(c):
            u = c % 2
            for h in range(NH):
                nc.tensor.transpose(oT_slots[h], o_sb[u][h][:], ident[:])
                with nc.allow_low_precision(reason="oT copy bf16"):
                    nc.vector.tensor_copy(oT_sb[u][h][:], oT_slots[h])

        def oproj(c):
            u = c % 2
            for dc in range(3):
                ops = op_ps[(c * 3 + dc) % 2]
                for h in range(NH):
                    nc.tensor.matmul(ops, oT_sb[u][h][:],
                                     wo_sb[:, h * D + dc * 512: h * D + (dc + 1) * 512],
                                     start=(h == 0), stop=(h == NH - 1))
                nc.scalar.activation(out_sb[u][:, dc * 512:(dc + 1) * 512], ops, _COPY)
            nc.sync.dma_start(out[c * P:(c + 1) * P, :], out_sb[u][:])

        def prep(c):
            """everything chunk c needs, interleaved for PE continuity."""
            proj_qk(c, 0)
            q2_trio(c, 0)
            vproj_third(c, 0)
            q2_trio(c, 1)
            vproj_third(c, 1)
            q2_trio(c, 2)
            proj_qk(c, 1)
            nc.sync.dma_start_transpose(k_rm[c % 2][:],
                                        kT_sb[:, c * P:(c + 1) * P])
            vproj_third(c, 2)
            fk2_mults(c)

        # ---- main loop
        prep(0)
        for c in range(NCH):
            attention(c)
            if c < NCH - 1:
                prep(c + 1)
            s_accum(c)
            normalize(c)
            if c > 0:
                oproj(c - 1)
            o_transpose(c)
        oproj(NCH - 1)

    nc.compile()
    return nc


def _host_inputs(hidden_states, Wq, Wk, Wv, Wo):
    """Shard + lay out the full inputs into 8 per-core in_maps."""
    mask = (np.arange(P)[:, None] <= np.arange(P)[None, :]).astype(np.float32)

    # replication selectors, copies at row offsets 0/32/64 (one per head base)
    rsel = np.zeros((GQ, 3 * P + 2), dtype=np.float32)
    for h in range(3):
        rsel[32 * h + FDIM, 3 * P + 0] = 2.0   # qT const-row bias
        rsel[32 * h + FDIM, 3 * P + 1] = 1.0   # kT const-row bias
    for r0 in (0, 32, 64):
        for i in range(8):
            for j in range(FDIM):
                rsel[r0 + j, FDIM * i + j] = 1.0            # rep <- q2[j]
                rsel[r0 + i, P + FDIM * i + j] = 1.0        # expA <- q2[i]
                rsel[r0 + 8 + i, 2 * P + FDIM * i + j] = 1.0  # expB <- q2[i+8]

    in_maps = []
    for core in range(8):
        b, g = divmod(core, 4)
        heads = range(NH * g, NH * (g + 1))
        wqk_pack = np.zeros((D, 2 * GQ), dtype=np.float32)
        for i, h in enumerate(heads):
            wqk_pack[:, 32 * i: 32 * i + FDIM] = Wq[:, FDIM * h: FDIM * (h + 1)] * 0.5
            wqk_pack[:, GQ + 32 * i: GQ + 32 * i + FDIM] = \
                Wk[:, FDIM * h: FDIM * (h + 1)]
        in_maps.append({
            "hsT": np.ascontiguousarray(hidden_states[b].T).astype(NPDT),
            "wqk": wqk_pack.astype(NPDT),
            "wv": np.ascontiguousarray(Wv[:, HD * NH * g: HD * NH * (g + 1)]).astype(NPDT),
            "wo": np.ascontiguousarray(Wo[HD * NH * g: HD * NH * (g + 1), :]).astype(NPDT),
            "maskd": mask.astype(NPDT),
            "rseld": rsel.astype(NPDT),
        })
    return in_maps


_NC = None


def _get_nc():
    global _NC
    if _NC is None:
        _NC = _build()
    return _NC


def run(hidden_states, Wq, Wk, Wv, Wo, trace=False, **trace_kwargs):
    nc = _get_nc()
    in_maps = _host_inputs(hidden_states, Wq, Wk, Wv, Wo)
    res = run_bass_kernel_spmd(nc, in_maps, core_ids=list(range(8)),
                               trace=trace, **trace_kwargs)
    out = np.zeros((B, L, D), dtype=np.float32)
    for core in range(8):
        out[core // 4] += res.results[core]["out"].astype(np.float32)
    return out, res


def kernel(hidden_states, Wq, Wk, Wv, Wo):
    out, _ = run(np.asarray(hidden_states, dtype=np.float32),
                 np.asarray(Wq, dtype=np.float32),
                 np.asarray(Wk, dtype=np.float32),
                 np.asarray(Wv, dtype=np.float32),
                 np.asarray(Wo, dtype=np.float32))
    return out
